# revision 30
# baseline (speedup 1.0000x reference)
"""Multi-Head Latent Attention (MLA) forward on 8 Trainium2 NeuronCores.

Sharding: tensor-parallel over heads (16 heads -> 2 per core). Each core:
  - computes q projections for its heads plus a 1/8 column slice of the
    latent-kv encoding from host-pretransposed xT; per-block AllGather
    assembles the full latent (hidden behind compute),
  - RMS-norms the latent (norm weight folded into wkv_b on host, per-token
    scale applied after the up-projection),
  - applies RoPE with host-precomputed cos/sin tables,
  - runs causal attention for its 2 heads in transposed-score layout
    (scores St[k, q]; softmax without max subtraction - scores are O(1));
    diagonal kt-blocks are processed on the live q-subrange only,
  - per-batch AllToAll exchanges head outputs so each core holds all
    features for a token slice, then computes that slice of the wo
    projection; batch 0's exchange + wo overlap batch 1's compute.
Output slices are disjoint; the host just concatenates them.

All matmul operands are bf16 (PE runs 1-pass vs fp32r's 2-pass HIGH mode);
PSUM accumulation stays fp32. wo is resident in SBUF (loaded once, bf16).
"""
import sys

if "/opt/trn_rl_repo" not in sys.path:
    sys.path.insert(0, "/opt/trn_rl_repo")

import numpy as np
import ml_dtypes
import concourse.bacc as bacc
import concourse.mybir as mybir
from concourse import tile
from concourse.masks import make_identity
from concourse.bass_utils import run_bass_kernel_spmd

H, NOPE, ROPE, VD, KVR, QKD = 16, 128, 64, 128, 512, 192
B, T, D = 2, 2048, 2048
NCORES, HPC, BLK = 8, 2, 512
KVC = KVR + ROPE  # 576 latent+rope columns
KVS = KVC // NCORES  # 72-column slice per core
W1N = HPC * QKD + KVS  # 456 projection columns per core
f32 = mybir.dt.float32
f32r = mybir.dt.float32r
bf16 = mybir.dt.bfloat16
EXP = mybir.ActivationFunctionType.Exp
LN = mybir.ActivationFunctionType.Ln
SQUARE = mybir.ActivationFunctionType.Square
BF16 = ml_dtypes.bfloat16


def _patch_act_tables():
    """Make the act-table-load pass serve Exp/Ln/Square from the one set that
    contains them all (natural_log_exp_and_others), so interleaved activations
    don't thrash table loads. Indices into act_info.json must be preserved, so
    the shadowing single-function sets are emptied in place, not removed."""
    import concourse.bacc as _bacc

    orig = _bacc.get_activation_tables
    if getattr(_bacc, "_mla_act_patch", False):
        return
    _bacc._mla_act_patch = True

    def patched(arch):
        d = dict(orig(arch))
        if "natural_log_exp_and_others" in d:
            for name in ("exp_and_others", "natural_log", "exp_and_friends"):
                if name in d:
                    d[name] = set()
        return d

    _bacc.get_activation_tables = patched


def build_program():
    _patch_act_tables()
    nc = bacc.Bacc("TRN2", target_bir_lowering=False, debug=False, num_devices=NCORES)
    xt_d = nc.dram_tensor("xt", [D, B * T], bf16, kind="ExternalInput")
    w1_d = nc.dram_tensor("w1", [D, W1N], bf16, kind="ExternalInput")
    wb_d = nc.dram_tensor("wb", [KVR, HPC * (NOPE + VD)], bf16, kind="ExternalInput")
    wo_d = nc.dram_tensor("wo", [H * VD, D], bf16, kind="ExternalInput")
    cos_d = nc.dram_tensor("cos", [128, T], bf16, kind="ExternalInput")
    sin_d = nc.dram_tensor("sin", [128, T], bf16, kind="ExternalInput")
    out_d = nc.dram_tensor("out", [B, T // NCORES, D], f32, kind="ExternalOutput")

    RG = [list(range(NCORES))]

    with tile.TileContext(nc) as tc:
        with (
            tc.tile_pool(name="dram", bufs=1, space="DRAM") as dram,
            tc.tile_pool(name="const", bufs=1) as const,
            tc.tile_pool(name="wpool", bufs=1) as wpool,
            tc.tile_pool(name="kvpool", bufs=1) as kvpool,
            tc.tile_pool(name="xtp", bufs=2) as xtp,
            tc.tile_pool(name="work", bufs=1) as work,
            tc.tile_pool(name="wop", bufs=1) as wop,
            tc.tile_pool(name="ps", bufs=1, space="PSUM") as ps,
        ):
            y_in = [
                [dram.tile([NCORES, VD, 256], bf16, name=f"y_in{b}_{h}")
                 for h in range(2)]
                for b in range(B)
            ]
            y_out = [
                [dram.tile([NCORES, VD, 256], bf16, name=f"y_out{b}_{h}")
                 for h in range(2)]
                for b in range(B)
            ]
            ag_in = [
                [dram.tile([KVS, BLK], bf16, name=f"ag_in{b}_{q}") for q in range(4)]
                for b in range(B)
            ]
            ag_out = [
                [dram.tile([KVC, BLK], bf16, name=f"ag_out{b}_{q}") for q in range(4)]
                for b in range(B)
            ]

            # dummy collective to absorb the ~25us cold-start of the first CC op
            warm_in = dram.tile([1, 128], bf16, name="warm_in")
            warm_out = dram.tile([NCORES, 128], bf16, name="warm_out")
            wsb = const.tile([1, 128], bf16, tag="warm")
            nc.gpsimd.memset(wsb[:], 0.0)
            nc.sync.dma_start(warm_in[:], wsb[:])
            nc.gpsimd.collective_compute(
                "AllGather",
                mybir.AluOpType.bypass,
                replica_groups=RG,
                ins=[warm_in.opt()],
                outs=[warm_out.opt()],
            )

            ident = const.tile([128, 128], bf16, tag="ident")
            make_identity(nc, ident)
            ones_f = const.tile([128, 1], f32, tag="ones_f")
            nc.gpsimd.memset(ones_f[:], 1.0)
            ones_r = const.tile([128, 1], f32r, tag="ones_r")
            nc.vector.tensor_copy(ones_r[:], ones_f[:])
            ones_b = const.tile([128, 1], bf16, tag="ones_b")
            nc.vector.tensor_copy(ones_b[:], ones_f[:])
            onesrow = const.tile([1, 128], bf16, tag="onesrow")
            nc.gpsimd.memset(onesrow[:], 1.0)
            eps = const.tile([1, 1], f32, tag="eps")
            nc.gpsimd.memset(eps[:], 1e-6)
            w1_sb = wpool.tile([128, 16, W1N], bf16, tag="w1")
            nc.sync.dma_start(w1_sb[:], w1_d[:].rearrange("(kc p) m -> p kc m", p=128))
            wb_sb = wpool.tile([128, 4, 512], bf16, tag="wb")
            nc.sync.dma_start(wb_sb[:], wb_d[:].rearrange("(kc p) m -> p kc m", p=128))
            wo_sb = wpool.tile([128, 16, D], bf16, tag="wo")
            nc.sync.dma_start(wo_sb[:], wo_d[:].rearrange("(kc p) m -> p kc m", p=128))

            # per-batch persistent kv staging (slots reused across batches)
            def alloc_kv():
                knope = [
                    kvpool.tile(
                        [NOPE, T], bf16, tag=f"knope{h}", bufs=1, name=f"knope{h}"
                    )
                    for h in range(2)
                ]
                vnat = [
                    kvpool.tile(
                        [128, 16, VD], bf16, tag=f"vnat{h}", bufs=1, name=f"vnat{h}"
                    )
                    for h in range(2)
                ]
                krope = kvpool.tile([ROPE, T], bf16, tag="krope", bufs=1)
                return knope, vnat, krope

            def stage_a1(b, qc):
                """q/kvc-slice projection from host-transposed xT + q rope + AG."""
                col0 = b * T + qc * BLK
                tok = slice(qc * BLK, (qc + 1) * BLK)
                csb = work.tile([128, BLK], bf16, tag="csb", bufs=3)
                ssb = work.tile([128, BLK], bf16, tag="ssb", bufs=3)
                nc.sync.dma_start(csb[:], cos_d[:, tok])
                nc.sync.dma_start(ssb[:], sin_d[:, tok])

                xT = xtp.tile([128, 16, BLK], bf16, tag="xT", bufs=2)
                nc.sync.dma_start(
                    xT[:],
                    xt_d[:, col0 : col0 + BLK].rearrange("(kc p) t -> p kc t", p=128),
                )

                qfT = work.tile([128, 4, BLK], bf16, tag="qfT", bufs=4)
                # mc chunk order: kvs slice first so the AllGather fires early
                for mc in (3, 0, 1, 2):
                    m0 = mc * 128
                    m1 = min(m0 + 128, W1N)
                    pp = ps.tile([128, BLK], f32, tag="proj", bufs=3, name="projp")
                    for kc in range(16):
                        nc.tensor.matmul(
                            pp[: m1 - m0, :],
                            w1_sb[:, kc, m0:m1],
                            xT[:, kc, :],
                            start=(kc == 0),
                            stop=(kc == 15),
                        )
                    if mc == 3:
                        # my kvc slice -> dram -> allgather
                        kvcm = work.tile([KVS, BLK], bf16, tag="kvcm", bufs=1)
                        nc.vector.tensor_copy(kvcm[:], pp[:KVS, :])
                        nc.sync.dma_start(ag_in[b][qc][:], kvcm[:])
                        nc.gpsimd.collective_compute(
                            "AllGather",
                            mybir.AluOpType.bypass,
                            replica_groups=RG,
                            ins=[ag_in[b][qc].opt()],
                            outs=[ag_out[b][qc].opt()],
                        )
                    elif mc < 2:
                        nc.vector.tensor_copy(qfT[:, mc, :], pp[:])
                    else:
                        # rope chunk: both heads' rope rows packed [h0 64 | h1 64]
                        rot = work.tile([128, BLK], bf16, tag="rot", bufs=2)
                        for hh in range(2):
                            r0 = hh * 64
                            nc.vector.tensor_scalar_mul(
                                rot[r0 : r0 + 32, :], pp[r0 + 32 : r0 + 64, :], -1.0
                            )
                            nc.vector.tensor_copy(
                                rot[r0 + 32 : r0 + 64, :], pp[r0 : r0 + 32, :]
                            )
                        nc.vector.tensor_mul(out=qfT[:, 2, :], in0=pp[:], in1=csb[:])
                        nc.vector.tensor_mul(out=rot[:], in0=rot[:], in1=ssb[:])
                        nc.vector.tensor_add(
                            out=qfT[:, 2, :], in0=qfT[:, 2, :], in1=rot[:]
                        )
                        # h1 roped rows 64:128 -> chunk 3 rows 0:64
                        nc.vector.tensor_copy(qfT[0:32, 3, :], qfT[64:96, 2, :])
                        nc.vector.tensor_copy(qfT[32:64, 3, :], qfT[96:128, 2, :])
                return qfT, csb, ssb

            def stage_a2(b, qc, knope, vnat, krope, csb, ssb):
                """post-AG: rms norm, kv up-projection, k rope."""
                tok = slice(qc * BLK, (qc + 1) * BLK)
                latent = work.tile([128, 4, BLK], bf16, tag="latent", bufs=1)
                nc.sync.dma_start(
                    latent[:],
                    ag_out[b][qc][:KVR, :].rearrange("(kc p) t -> p kc t", p=128),
                )
                kraw = work.tile([ROPE, BLK], bf16, tag="kraw", bufs=2)
                nc.sync.dma_start(kraw[:], ag_out[b][qc][KVR:, :])

                # sum of squares over latent dims (ACT square + PE ones-mm)
                ssq = ps.tile([1, BLK], f32, tag="xps", bufs=1, name="ssq")
                for i in range(4):
                    sqc = work.tile([128, BLK], bf16, tag="sqc", bufs=1)
                    nc.scalar.activation(sqc[:], latent[:, i, :], SQUARE)
                    nc.tensor.matmul(
                        ssq[:], ones_b[:], sqc[:], start=(i == 0), stop=(i == 3)
                    )
                # rms scale: 1/sqrt(ssq/512+eps) = exp(-0.5*ln(.))
                lnrow = work.tile([1, BLK], f32, tag="lnrow", bufs=2)
                nc.scalar.activation(lnrow[:], ssq[:], LN, bias=eps[:], scale=1.0 / KVR)
                invrow = work.tile([1, BLK], bf16, tag="invrow", bufs=2)
                nc.scalar.activation(invrow[:], lnrow[:], EXP, scale=-0.5)
                invbc_ps = ps.tile([128, BLK], f32, tag="xps", bufs=1, name="invbc_ps")
                nc.tensor.matmul(invbc_ps[:], onesrow[:], invrow[:])
                invbc = work.tile([128, BLK], bf16, tag="invbc", bufs=2)
                nc.vector.tensor_copy(invbc[:], invbc_ps[:])

                # k rope from gathered raw rows
                rot = work.tile([128, BLK], bf16, tag="rot", bufs=2)
                kr = krope[:, tok]
                nc.vector.tensor_scalar_mul(rot[0:32, :], kraw[32:64, :], -1.0)
                nc.vector.tensor_copy(rot[32:64, :], kraw[0:32, :])
                nc.vector.tensor_mul(out=kr, in0=kraw[:], in1=csb[0:64, :])
                nc.vector.tensor_mul(
                    out=rot[0:64, :], in0=rot[0:64, :], in1=ssb[0:64, :]
                )
                nc.vector.tensor_add(out=kr, in0=kr, in1=rot[0:64, :])

                # kv up-projection + normalize; v transposed to natural
                for mc in range(4):  # [h0 nope, h0 v, h1 nope, h1 v]
                    h = mc // 2
                    pkv = ps.tile([128, BLK], f32, tag="proj", bufs=3)
                    for kc in range(4):
                        nc.tensor.matmul(
                            pkv[:],
                            wb_sb[:, kc, mc * 128 : (mc + 1) * 128],
                            latent[:, kc, :],
                            start=(kc == 0),
                            stop=(kc == 3),
                        )
                    if mc % 2 == 0:
                        nc.vector.tensor_mul(
                            out=knope[h][:, tok], in0=pkv[:], in1=invbc[:]
                        )
                    else:
                        vuT = work.tile([128, BLK], bf16, tag="vuT", bufs=1)
                        nc.vector.tensor_mul(out=vuT[:], in0=pkv[:], in1=invbc[:])
                        pvt = ps.tile([128, BLK], bf16, tag="xps", bufs=1)
                        for tt in range(4):
                            nc.tensor.transpose(
                                pvt[:, tt * 128 : (tt + 1) * 128],
                                vuT[:, tt * 128 : (tt + 1) * 128],
                                ident[:],
                            )
                        for tt in range(4):
                            nc.vector.tensor_copy(
                                vnat[h][:, qc * 4 + tt, :],
                                pvt[:, tt * 128 : (tt + 1) * 128],
                            )

            def stage_b(b, qc, qfT, knope, vnat, krope, mid=None):
                """causal attention for one q-chunk, both heads.

                Diagonal kt-blocks only touch the live q-subrange
                [128*kt_rel, 512); fully-masked columns are never computed."""
                n_kt = 4 * (qc + 1)
                for h in range(2):
                    yacc = ps.tile([VD, BLK], f32, tag="yacc", bufs=1)
                    acc_e = work.tile([128, BLK], f32r, tag="acc_e", bufs=1)
                    acc_o = work.tile([128, BLK], f32r, tag="acc_o", bufs=1)
                    qrope = qfT[0:64, 2 + h, :]
                    for kt in range(n_kt):
                        ks = slice(kt * 128, (kt + 1) * 128)
                        kt_rel = kt - 4 * qc
                        qs = max(0, 128 * kt_rel)  # live q-subrange start
                        qr = slice(qs, BLK)
                        st = ps.tile([128, BLK], f32, tag="st", bufs=3)
                        nc.tensor.matmul(
                            st[:, qr],
                            knope[h][:, ks],
                            qfT[:, h, qr],
                            start=True,
                            stop=False,
                        )
                        nc.tensor.matmul(
                            st[:, qr], krope[:, ks], qrope[:, qr],
                            start=False, stop=True,
                        )
                        est = work.tile([128, BLK], bf16, tag="est", bufs=3)
                        nc.scalar.activation(est[:, qr], st[:, qr], EXP)
                        if kt_rel >= 0:
                            # causal triangle only in the first 128 live cols
                            nc.gpsimd.affine_select(
                                out=est[:, qs : qs + 128],
                                in_=est[:, qs : qs + 128],
                                compare_op=mybir.AluOpType.is_ge,
                                fill=0.0,
                                base=0,
                                pattern=[[1, 128]],
                                channel_multiplier=-1,
                            )
                        nc.tensor.matmul(
                            yacc[:, qr],
                            vnat[h][:, kt, :],
                            est[:, qr],
                            start=(kt == 0),
                            stop=(kt == n_kt - 1),
                        )
                        if kt == 0:
                            nc.vector.tensor_copy(acc_e[:], est[:])
                        elif qc == 0:
                            # smallest chunk: single accumulator, alternate engines
                            eng = nc.vector if kt % 2 == 0 else nc.gpsimd
                            eng.tensor_add(
                                out=acc_e[:, qr], in0=acc_e[:, qr], in1=est[:, qr]
                            )
                        elif kt == 1:
                            # full width for qc>0, so acc_o init needs no masking
                            nc.gpsimd.tensor_copy(acc_o[:], est[:])
                        elif kt % 2 == 0:
                            nc.vector.tensor_add(
                                out=acc_e[:, qr], in0=acc_e[:, qr], in1=est[:, qr]
                            )
                        else:
                            nc.gpsimd.tensor_add(
                                out=acc_o[:, qr], in0=acc_o[:, qr], in1=est[:, qr]
                            )

                    sums = ps.tile([1, BLK], f32, tag="st", bufs=3)
                    if qc > 0:
                        nc.tensor.matmul(
                            sums[:], ones_r[:], acc_e[:], start=True, stop=False
                        )
                        nc.tensor.matmul(
                            sums[:], ones_r[:], acc_o[:], start=False, stop=True
                        )
                    else:
                        nc.tensor.matmul(sums[:], ones_r[:], acc_e[:])
                    lnr = work.tile([1, BLK], f32, tag="lnrow", bufs=2)
                    nc.scalar.activation(lnr[:], sums[:], LN)
                    sinvrow = work.tile([1, BLK], bf16, tag="invrow", bufs=2)
                    nc.scalar.activation(sinvrow[:], lnr[:], EXP, scale=-1.0)
                    sbc_ps = ps.tile([128, BLK], f32, tag="st", bufs=3, name="sbc_ps")
                    nc.tensor.matmul(sbc_ps[:], onesrow[:], sinvrow[:])
                    sinv = work.tile([128, BLK], f32, tag="sinv", bufs=1)
                    nc.vector.tensor_copy(sinv[:], sbc_ps[:])
                    ysb = work.tile([VD, BLK], bf16, tag="ysb", bufs=2)
                    nc.vector.tensor_mul(out=ysb[:], in0=yacc[:], in1=sinv[:])
                    for jj in range(2):
                        nc.sync.dma_start(
                            y_in[b][h][qc * 2 + jj, :, :],
                            ysb[:, jj * 256 : (jj + 1) * 256],
                        )
                    if mid is not None and h == 0:
                        mid()

            def emit_a2a(b, h):
                nc.gpsimd.collective_compute(
                    "AllToAll",
                    mybir.AluOpType.bypass,
                    replica_groups=RG,
                    ins=[y_in[b][h].opt()],
                    outs=[y_out[b][h].opt()],
                )

            def emit_wo(b):
                """wo projection for this batch's gathered token slice."""
                a2a = wop.tile([128, 16, 256], bf16, tag="a2a", bufs=1, name="a2a")
                kc_order = [2 * s for s in range(8)] + [2 * s + 1 for s in range(8)]
                for kc in kc_order:
                    nc.sync.dma_start(
                        a2a[:, kc, :], y_out[b][kc % 2][kc // 2, :, :]
                    )
                for n in range(4):
                    pouts = [
                        ps.tile([128, 512], f32, tag="st", bufs=3, name="outp")
                        for _ in range(2)
                    ]
                    for ki, kc in enumerate(kc_order):
                        for tt in range(2):
                            nc.tensor.matmul(
                                pouts[tt][:],
                                a2a[:, kc, tt * 128 : (tt + 1) * 128],
                                wo_sb[:, kc, n * 512 : (n + 1) * 512],
                                start=(ki == 0),
                                stop=(ki == 15),
                            )
                    for tt in range(2):
                        osb = wop.tile([128, 512], f32, tag="osb", bufs=2)
                        nc.vector.tensor_copy(osb[:], pouts[tt][:])
                        nc.sync.dma_start(
                            out_d[
                                b, tt * 128 : (tt + 1) * 128, n * 512 : (n + 1) * 512
                            ],
                            osb[:],
                        )

            # ---- software-pipelined schedule ----
            for b in range(B):
                knope, vnat, krope = alloc_kv()
                st_a = {}
                st_a[0] = stage_a1(b, 0)
                st_a[1] = stage_a1(b, 1)
                st_a[2] = stage_a1(b, 2)
                stage_a2(b, 0, knope, vnat, krope, st_a[0][1], st_a[0][2])
                st_a[3] = stage_a1(b, 3)
                stage_a2(b, 1, knope, vnat, krope, st_a[1][1], st_a[1][2])
                stage_b(b, 0, st_a[0][0], knope, vnat, krope)
                stage_a2(b, 2, knope, vnat, krope, st_a[2][1], st_a[2][2])
                stage_b(b, 1, st_a[1][0], knope, vnat, krope)
                stage_a2(b, 3, knope, vnat, krope, st_a[3][1], st_a[3][2])
                stage_b(b, 2, st_a[2][0], knope, vnat, krope)
                stage_b(
                    b, 3, st_a[3][0], knope, vnat, krope,
                    mid=lambda b=b: emit_a2a(b, 0),
                )
                emit_a2a(b, 1)
                emit_wo(b)

    nc.compile()
    return nc


def host_prep(x, wq, wkv_a, wkv_b, wo, kv_norm_w):
    scale = np.float32(QKD ** -0.5)
    inv = (1.0 / (10000.0 ** (np.arange(0, ROPE, 2, dtype=np.float32) / ROPE))).astype(
        np.float32
    )
    f = np.outer(np.arange(T, dtype=np.float32), inv)
    cos32 = np.cos(f).T.astype(np.float32)
    sin32 = np.sin(f).T.astype(np.float32)
    cos128 = np.ascontiguousarray(np.concatenate([cos32] * 4, 0)).astype(BF16)
    sin128 = np.ascontiguousarray(np.concatenate([sin32] * 4, 0)).astype(BF16)
    wkv_bw = (wkv_b * kv_norm_w[:, None]).astype(np.float32)
    xt = np.ascontiguousarray(x.reshape(B * T, D).astype(BF16).T)
    wo_c = np.ascontiguousarray(wo).astype(BF16)
    wq_r = wq.reshape(D, H, QKD)

    in_maps = []
    for c in range(NCORES):
        h0 = HPC * c
        w1 = np.concatenate(
            [
                wq_r[:, h0, :NOPE] * scale,
                wq_r[:, h0 + 1, :NOPE] * scale,
                wq_r[:, h0, NOPE:] * scale,
                wq_r[:, h0 + 1, NOPE:] * scale,
                wkv_a[:, c * KVS : (c + 1) * KVS],
            ],
            axis=1,
        ).astype(BF16)
        wb = np.ascontiguousarray(
            wkv_bw[:, h0 * (NOPE + VD) : (h0 + 2) * (NOPE + VD)]
        ).astype(BF16)
        in_maps.append(
            {
                "xt": xt,
                "w1": np.ascontiguousarray(w1),
                "wb": wb,
                "wo": wo_c,
                "cos": cos128,
                "sin": sin128,
            }
        )
    return in_maps


_NC = None


def kernel(x, wq, wkv_a, wkv_b, wo, kv_norm_w, _trace=False):
    global _NC
    if _NC is None:
        _NC = build_program()
    in_maps = host_prep(
        np.asarray(x, np.float32),
        np.asarray(wq, np.float32),
        np.asarray(wkv_a, np.float32),
        np.asarray(wkv_b, np.float32),
        np.asarray(wo, np.float32),
        np.asarray(kv_norm_w, np.float32),
    )
    res = run_bass_kernel_spmd(_NC, in_maps, list(range(NCORES)), trace=_trace)
    out = np.empty((B, T, D), np.float32)
    cw = T // NCORES
    for c in range(NCORES):
        oc = res.results[c]["out"]  # (B, 256, D)
        for b in range(B):
            out[b, c * cw : (c + 1) * cw, :] = oc[b]
    kernel.last_results = res
    return out


# revision 31
# speedup vs baseline: 1.0554x; 1.0554x over previous
"""Multi-Head Latent Attention (MLA) forward on 8 Trainium2 NeuronCores.

Sharding: tensor-parallel over heads (16 heads -> 2 per core). Each core:
  - computes q projections for its heads plus a 1/8 column slice of the
    latent-kv encoding from host-pretransposed xT; per-block AllGather
    assembles the full latent (hidden behind compute),
  - RMS-norms the latent (norm weight folded into wkv_b on host, per-token
    scale applied after the up-projection),
  - applies RoPE with host-precomputed cos/sin tables,
  - runs causal attention for its 2 heads in transposed-score layout
    (scores St[k, q]; softmax without max subtraction - scores are O(1));
    diagonal kt-blocks are processed on the live q-subrange only,
  - per-batch AllToAll exchanges head outputs so each core holds all
    features for a token slice, then computes that slice of the wo
    projection; batch 0's exchange + wo overlap batch 1's compute.
Output slices are disjoint; the host just concatenates them.

All matmul operands are bf16 (PE runs 1-pass vs fp32r's 2-pass HIGH mode);
PSUM accumulation stays fp32. wo is resident in SBUF (loaded once, bf16).
"""
import sys

if "/opt/trn_rl_repo" not in sys.path:
    sys.path.insert(0, "/opt/trn_rl_repo")

import numpy as np
import ml_dtypes
import concourse.bacc as bacc
import concourse.mybir as mybir
from concourse import tile
from concourse.masks import make_identity
from concourse.bass_utils import run_bass_kernel_spmd

H, NOPE, ROPE, VD, KVR, QKD = 16, 128, 64, 128, 512, 192
B, T, D = 2, 2048, 2048
NCORES, HPC, BLK = 8, 2, 512
KVC = KVR + ROPE  # 576 latent+rope columns
KVS = KVC // NCORES  # 72-column slice per core
W1N = HPC * QKD + KVS  # 456 projection columns per core
f32 = mybir.dt.float32
f32r = mybir.dt.float32r
bf16 = mybir.dt.bfloat16
EXP = mybir.ActivationFunctionType.Exp
LN = mybir.ActivationFunctionType.Ln
SQUARE = mybir.ActivationFunctionType.Square
BF16 = ml_dtypes.bfloat16


def _patch_act_tables():
    """Make the act-table-load pass serve Exp/Ln/Square from the one set that
    contains them all (natural_log_exp_and_others), so interleaved activations
    don't thrash table loads. Indices into act_info.json must be preserved, so
    the shadowing single-function sets are emptied in place, not removed."""
    import concourse.bacc as _bacc

    orig = _bacc.get_activation_tables
    if getattr(_bacc, "_mla_act_patch", False):
        return
    _bacc._mla_act_patch = True

    def patched(arch):
        d = dict(orig(arch))
        if "natural_log_exp_and_others" in d:
            for name in ("exp_and_others", "natural_log", "exp_and_friends"):
                if name in d:
                    d[name] = set()
        return d

    _bacc.get_activation_tables = patched


def build_program():
    _patch_act_tables()
    nc = bacc.Bacc("TRN2", target_bir_lowering=False, debug=False, num_devices=NCORES)
    xt_d = nc.dram_tensor("xt", [D, B * T], bf16, kind="ExternalInput")
    w1_d = nc.dram_tensor("w1", [D, W1N], bf16, kind="ExternalInput")
    wb_d = nc.dram_tensor("wb", [KVR, HPC * (NOPE + VD)], bf16, kind="ExternalInput")
    wo_d = nc.dram_tensor("wo", [H * VD, D], bf16, kind="ExternalInput")
    cos_d = nc.dram_tensor("cos", [128, T], bf16, kind="ExternalInput")
    sin_d = nc.dram_tensor("sin", [128, T], bf16, kind="ExternalInput")
    out_d = nc.dram_tensor("out", [B, T // NCORES, D], f32, kind="ExternalOutput")

    RG = [list(range(NCORES))]

    with tile.TileContext(nc) as tc:
        with (
            tc.tile_pool(name="dram", bufs=1, space="DRAM") as dram,
            tc.tile_pool(name="const", bufs=1) as const,
            tc.tile_pool(name="wpool", bufs=1) as wpool,
            tc.tile_pool(name="kvpool", bufs=1) as kvpool,
            tc.tile_pool(name="xtp", bufs=2) as xtp,
            tc.tile_pool(name="work", bufs=1) as work,
            tc.tile_pool(name="wop", bufs=1) as wop,
            tc.tile_pool(name="ps", bufs=1, space="PSUM") as ps,
        ):
            y_in = [
                [dram.tile([NCORES, VD, 256], bf16, name=f"y_in{b}_{h}")
                 for h in range(2)]
                for b in range(B)
            ]
            y_out = [
                [dram.tile([NCORES, VD, 256], bf16, name=f"y_out{b}_{h}")
                 for h in range(2)]
                for b in range(B)
            ]
            ag_in = [
                [dram.tile([KVS, BLK], bf16, name=f"ag_in{b}_{q}") for q in range(4)]
                for b in range(B)
            ]
            ag_out = [
                [dram.tile([KVC, BLK], bf16, name=f"ag_out{b}_{q}") for q in range(4)]
                for b in range(B)
            ]

            # dummy collective to absorb the ~25us cold-start of the first CC op
            warm_in = dram.tile([1, 128], bf16, name="warm_in")
            warm_out = dram.tile([NCORES, 128], bf16, name="warm_out")
            wsb = const.tile([1, 128], bf16, tag="warm")
            nc.gpsimd.memset(wsb[:], 0.0)
            nc.sync.dma_start(warm_in[:], wsb[:])
            nc.gpsimd.collective_compute(
                "AllGather",
                mybir.AluOpType.bypass,
                replica_groups=RG,
                ins=[warm_in.opt()],
                outs=[warm_out.opt()],
            )

            ident = const.tile([128, 128], bf16, tag="ident")
            make_identity(nc, ident)
            ones_f = const.tile([128, 1], f32, tag="ones_f")
            nc.gpsimd.memset(ones_f[:], 1.0)
            ones_r = const.tile([128, 1], f32r, tag="ones_r")
            nc.vector.tensor_copy(ones_r[:], ones_f[:])
            ones_b = const.tile([128, 1], bf16, tag="ones_b")
            nc.vector.tensor_copy(ones_b[:], ones_f[:])
            onesrow = const.tile([1, 128], bf16, tag="onesrow")
            nc.gpsimd.memset(onesrow[:], 1.0)
            eps = const.tile([1, 1], f32, tag="eps")
            nc.gpsimd.memset(eps[:], 1e-6)
            w1_sb = wpool.tile([128, 16, W1N], bf16, tag="w1")
            nc.sync.dma_start(w1_sb[:], w1_d[:].rearrange("(kc p) m -> p kc m", p=128))
            wb_sb = wpool.tile([128, 4, 512], bf16, tag="wb")
            nc.sync.dma_start(wb_sb[:], wb_d[:].rearrange("(kc p) m -> p kc m", p=128))
            wo_sb = wpool.tile([128, 16, D], bf16, tag="wo")
            nc.sync.dma_start(wo_sb[:], wo_d[:].rearrange("(kc p) m -> p kc m", p=128))

            # per-batch persistent kv staging (slots reused across batches)
            def alloc_kv():
                knope = [
                    kvpool.tile(
                        [NOPE, T], bf16, tag=f"knope{h}", bufs=1, name=f"knope{h}"
                    )
                    for h in range(2)
                ]
                vnat = [
                    kvpool.tile(
                        [128, 16, VD], bf16, tag=f"vnat{h}", bufs=1, name=f"vnat{h}"
                    )
                    for h in range(2)
                ]
                krope = kvpool.tile([ROPE, T], bf16, tag="krope", bufs=1)
                return knope, vnat, krope

            def stage_a1(b, qc):
                """q/kvc-slice projection from host-transposed xT + q rope + AG."""
                col0 = b * T + qc * BLK
                tok = slice(qc * BLK, (qc + 1) * BLK)
                csb = work.tile([128, BLK], bf16, tag="csb", bufs=3)
                ssb = work.tile([128, BLK], bf16, tag="ssb", bufs=3)
                nc.sync.dma_start(csb[:], cos_d[:, tok])
                nc.sync.dma_start(ssb[:], sin_d[:, tok])

                xT = xtp.tile([128, 16, BLK], bf16, tag="xT", bufs=2)
                nc.sync.dma_start(
                    xT[:],
                    xt_d[:, col0 : col0 + BLK].rearrange("(kc p) t -> p kc t", p=128),
                )

                qfT = work.tile([128, 4, BLK], bf16, tag="qfT", bufs=4)
                # mc chunk order: kvs slice first so the AllGather fires early
                for mc in (3, 0, 1, 2):
                    m0 = mc * 128
                    m1 = min(m0 + 128, W1N)
                    pp = ps.tile([128, BLK], f32, tag="proj", bufs=2, name="projp")
                    for kc in range(16):
                        nc.tensor.matmul(
                            pp[: m1 - m0, :],
                            w1_sb[:, kc, m0:m1],
                            xT[:, kc, :],
                            start=(kc == 0),
                            stop=(kc == 15),
                        )
                    if mc == 3:
                        # my kvc slice -> dram -> allgather
                        kvcm = work.tile([KVS, BLK], bf16, tag="kvcm", bufs=1)
                        nc.vector.tensor_copy(kvcm[:], pp[:KVS, :])
                        nc.sync.dma_start(ag_in[b][qc][:], kvcm[:])
                        nc.gpsimd.collective_compute(
                            "AllGather",
                            mybir.AluOpType.bypass,
                            replica_groups=RG,
                            ins=[ag_in[b][qc].opt()],
                            outs=[ag_out[b][qc].opt()],
                        )
                    elif mc < 2:
                        nc.vector.tensor_copy(qfT[:, mc, :], pp[:])
                    else:
                        # rope chunk: both heads' rope rows packed [h0 64 | h1 64]
                        rot = work.tile([128, BLK], bf16, tag="rot", bufs=2)
                        for hh in range(2):
                            r0 = hh * 64
                            nc.vector.tensor_scalar_mul(
                                rot[r0 : r0 + 32, :], pp[r0 + 32 : r0 + 64, :], -1.0
                            )
                            nc.vector.tensor_copy(
                                rot[r0 + 32 : r0 + 64, :], pp[r0 : r0 + 32, :]
                            )
                        nc.vector.tensor_mul(out=qfT[:, 2, :], in0=pp[:], in1=csb[:])
                        nc.vector.tensor_mul(out=rot[:], in0=rot[:], in1=ssb[:])
                        nc.vector.tensor_add(
                            out=qfT[:, 2, :], in0=qfT[:, 2, :], in1=rot[:]
                        )
                        # h1 roped rows 64:128 -> chunk 3 rows 0:64
                        nc.vector.tensor_copy(qfT[0:32, 3, :], qfT[64:96, 2, :])
                        nc.vector.tensor_copy(qfT[32:64, 3, :], qfT[96:128, 2, :])
                return qfT, csb, ssb

            def stage_a2(b, qc, knope, vnat, krope, csb, ssb):
                """post-AG: rms norm, kv up-projection, k rope."""
                tok = slice(qc * BLK, (qc + 1) * BLK)
                latent = work.tile([128, 4, BLK], bf16, tag="latent", bufs=1)
                nc.sync.dma_start(
                    latent[:],
                    ag_out[b][qc][:KVR, :].rearrange("(kc p) t -> p kc t", p=128),
                )
                kraw = work.tile([ROPE, BLK], bf16, tag="kraw", bufs=2)
                nc.sync.dma_start(kraw[:], ag_out[b][qc][KVR:, :])

                # sum of squares over latent dims (ACT square + PE ones-mm)
                ssq = ps.tile([1, BLK], f32, tag="xps", bufs=1, name="ssq")
                for i in range(4):
                    sqc = work.tile([128, BLK], bf16, tag="sqc", bufs=1)
                    nc.scalar.activation(sqc[:], latent[:, i, :], SQUARE)
                    nc.tensor.matmul(
                        ssq[:], ones_b[:], sqc[:], start=(i == 0), stop=(i == 3)
                    )
                # rms scale: 1/sqrt(ssq/512+eps) = exp(-0.5*ln(.))
                lnrow = work.tile([1, BLK], f32, tag="lnrow", bufs=2)
                nc.scalar.activation(lnrow[:], ssq[:], LN, bias=eps[:], scale=1.0 / KVR)
                invrow = work.tile([1, BLK], bf16, tag="invrow", bufs=2)
                nc.scalar.activation(invrow[:], lnrow[:], EXP, scale=-0.5)
                invbc_ps = ps.tile([128, BLK], f32, tag="xps", bufs=1, name="invbc_ps")
                nc.tensor.matmul(invbc_ps[:], onesrow[:], invrow[:])
                invbc = work.tile([128, BLK], bf16, tag="invbc", bufs=2)
                nc.vector.tensor_copy(invbc[:], invbc_ps[:])

                # k rope from gathered raw rows
                rot = work.tile([128, BLK], bf16, tag="rot", bufs=2)
                kr = krope[:, tok]
                nc.vector.tensor_scalar_mul(rot[0:32, :], kraw[32:64, :], -1.0)
                nc.vector.tensor_copy(rot[32:64, :], kraw[0:32, :])
                nc.vector.tensor_mul(out=kr, in0=kraw[:], in1=csb[0:64, :])
                nc.vector.tensor_mul(
                    out=rot[0:64, :], in0=rot[0:64, :], in1=ssb[0:64, :]
                )
                nc.vector.tensor_add(out=kr, in0=kr, in1=rot[0:64, :])

                # kv up-projection + normalize; v transposed to natural
                for mc in range(4):  # [h0 nope, h0 v, h1 nope, h1 v]
                    h = mc // 2
                    pkv = ps.tile([128, BLK], f32, tag="proj", bufs=2)
                    for kc in range(4):
                        nc.tensor.matmul(
                            pkv[:],
                            wb_sb[:, kc, mc * 128 : (mc + 1) * 128],
                            latent[:, kc, :],
                            start=(kc == 0),
                            stop=(kc == 3),
                        )
                    if mc % 2 == 0:
                        nc.vector.tensor_mul(
                            out=knope[h][:, tok], in0=pkv[:], in1=invbc[:]
                        )
                    else:
                        vuT = work.tile([128, BLK], bf16, tag="vuT", bufs=1)
                        nc.vector.tensor_mul(out=vuT[:], in0=pkv[:], in1=invbc[:])
                        pvt = ps.tile([128, BLK], bf16, tag="xps", bufs=1)
                        for tt in range(4):
                            nc.tensor.transpose(
                                pvt[:, tt * 128 : (tt + 1) * 128],
                                vuT[:, tt * 128 : (tt + 1) * 128],
                                ident[:],
                            )
                        for tt in range(4):
                            nc.vector.tensor_copy(
                                vnat[h][:, qc * 4 + tt, :],
                                pvt[:, tt * 128 : (tt + 1) * 128],
                            )

            def stage_b(b, qc, qfT, knope, vnat, krope, mid=None):
                """causal attention for one q-chunk, both heads.

                Diagonal kt-blocks only touch the live q-subrange
                [128*kt_rel, 512); fully-masked columns are never computed."""
                n_kt = 4 * (qc + 1)
                for h in range(2):
                    yacc = ps.tile([VD, BLK], f32, tag="yacc", bufs=2)
                    acc_e = work.tile([128, BLK], f32r, tag="acc_e", bufs=1)
                    acc_o = work.tile([128, BLK], f32r, tag="acc_o", bufs=1)
                    qrope = qfT[0:64, 2 + h, :]
                    for kt in range(n_kt):
                        ks = slice(kt * 128, (kt + 1) * 128)
                        kt_rel = kt - 4 * qc
                        qs = max(0, 128 * kt_rel)  # live q-subrange start
                        qr = slice(qs, BLK)
                        st = ps.tile([128, BLK], f32, tag="st", bufs=3)
                        nc.tensor.matmul(
                            st[:, qr],
                            knope[h][:, ks],
                            qfT[:, h, qr],
                            start=True,
                            stop=False,
                        )
                        nc.tensor.matmul(
                            st[:, qr], krope[:, ks], qrope[:, qr],
                            start=False, stop=True,
                        )
                        est = work.tile([128, BLK], bf16, tag="est", bufs=3)
                        nc.scalar.activation(est[:, qr], st[:, qr], EXP)
                        if kt_rel >= 0:
                            # causal triangle only in the first 128 live cols
                            nc.gpsimd.affine_select(
                                out=est[:, qs : qs + 128],
                                in_=est[:, qs : qs + 128],
                                compare_op=mybir.AluOpType.is_ge,
                                fill=0.0,
                                base=0,
                                pattern=[[1, 128]],
                                channel_multiplier=-1,
                            )
                        nc.tensor.matmul(
                            yacc[:, qr],
                            vnat[h][:, kt, :],
                            est[:, qr],
                            start=(kt == 0),
                            stop=(kt == n_kt - 1),
                        )
                        if kt == 0:
                            nc.vector.tensor_copy(acc_e[:], est[:])
                        elif qc == 0:
                            # smallest chunk: single accumulator, alternate engines
                            eng = nc.vector if kt % 2 == 0 else nc.gpsimd
                            eng.tensor_add(
                                out=acc_e[:, qr], in0=acc_e[:, qr], in1=est[:, qr]
                            )
                        elif kt == 1:
                            # full width for qc>0, so acc_o init needs no masking
                            nc.gpsimd.tensor_copy(acc_o[:], est[:])
                        elif kt % 2 == 0:
                            nc.vector.tensor_add(
                                out=acc_e[:, qr], in0=acc_e[:, qr], in1=est[:, qr]
                            )
                        else:
                            nc.gpsimd.tensor_add(
                                out=acc_o[:, qr], in0=acc_o[:, qr], in1=est[:, qr]
                            )

                    sums = ps.tile([1, BLK], f32, tag="st", bufs=3)
                    if qc > 0:
                        nc.tensor.matmul(
                            sums[:], ones_r[:], acc_e[:], start=True, stop=False
                        )
                        nc.tensor.matmul(
                            sums[:], ones_r[:], acc_o[:], start=False, stop=True
                        )
                    else:
                        nc.tensor.matmul(sums[:], ones_r[:], acc_e[:])
                    lnr = work.tile([1, BLK], f32, tag="lnrow", bufs=2)
                    nc.scalar.activation(lnr[:], sums[:], LN)
                    sinvrow = work.tile([1, BLK], bf16, tag="invrow", bufs=2)
                    nc.scalar.activation(sinvrow[:], lnr[:], EXP, scale=-1.0)
                    sbc_ps = ps.tile([128, BLK], f32, tag="st", bufs=3, name="sbc_ps")
                    nc.tensor.matmul(sbc_ps[:], onesrow[:], sinvrow[:])
                    sinv = work.tile([128, BLK], f32, tag="sinv", bufs=1)
                    nc.vector.tensor_copy(sinv[:], sbc_ps[:])
                    ysb = work.tile([VD, BLK], bf16, tag="ysb", bufs=2)
                    nc.vector.tensor_mul(out=ysb[:], in0=yacc[:], in1=sinv[:])
                    for jj in range(2):
                        nc.sync.dma_start(
                            y_in[b][h][qc * 2 + jj, :, :],
                            ysb[:, jj * 256 : (jj + 1) * 256],
                        )
                    if mid is not None and h == 0:
                        mid()

            def emit_a2a(b, h):
                nc.gpsimd.collective_compute(
                    "AllToAll",
                    mybir.AluOpType.bypass,
                    replica_groups=RG,
                    ins=[y_in[b][h].opt()],
                    outs=[y_out[b][h].opt()],
                )

            def emit_wo(b):
                """wo projection for this batch's gathered token slice."""
                a2a = wop.tile([128, 16, 256], bf16, tag="a2a", bufs=1, name="a2a")
                kc_order = [2 * s for s in range(8)] + [2 * s + 1 for s in range(8)]
                for kc in kc_order:
                    nc.sync.dma_start(
                        a2a[:, kc, :], y_out[b][kc % 2][kc // 2, :, :]
                    )
                for n in range(4):
                    pouts = [
                        ps.tile([128, 512], f32, tag="st", bufs=3, name="outp")
                        for _ in range(2)
                    ]
                    for ki, kc in enumerate(kc_order):
                        for tt in range(2):
                            nc.tensor.matmul(
                                pouts[tt][:],
                                a2a[:, kc, tt * 128 : (tt + 1) * 128],
                                wo_sb[:, kc, n * 512 : (n + 1) * 512],
                                start=(ki == 0),
                                stop=(ki == 15),
                            )
                    for tt in range(2):
                        osb = wop.tile([128, 512], f32, tag="osb", bufs=2)
                        nc.vector.tensor_copy(osb[:], pouts[tt][:])
                        nc.sync.dma_start(
                            out_d[
                                b, tt * 128 : (tt + 1) * 128, n * 512 : (n + 1) * 512
                            ],
                            osb[:],
                        )

            # ---- software-pipelined schedule ----
            for b in range(B):
                knope, vnat, krope = alloc_kv()
                st_a = {}
                st_a[0] = stage_a1(b, 0)
                st_a[1] = stage_a1(b, 1)
                st_a[2] = stage_a1(b, 2)
                stage_a2(b, 0, knope, vnat, krope, st_a[0][1], st_a[0][2])
                st_a[3] = stage_a1(b, 3)
                stage_a2(b, 1, knope, vnat, krope, st_a[1][1], st_a[1][2])
                stage_b(b, 0, st_a[0][0], knope, vnat, krope)
                stage_a2(b, 2, knope, vnat, krope, st_a[2][1], st_a[2][2])
                stage_b(b, 1, st_a[1][0], knope, vnat, krope)
                stage_a2(b, 3, knope, vnat, krope, st_a[3][1], st_a[3][2])
                stage_b(b, 2, st_a[2][0], knope, vnat, krope)
                stage_b(
                    b, 3, st_a[3][0], knope, vnat, krope,
                    mid=lambda b=b: emit_a2a(b, 0),
                )
                emit_a2a(b, 1)
                emit_wo(b)

    nc.compile()
    return nc


def host_prep(x, wq, wkv_a, wkv_b, wo, kv_norm_w):
    scale = np.float32(QKD ** -0.5)
    inv = (1.0 / (10000.0 ** (np.arange(0, ROPE, 2, dtype=np.float32) / ROPE))).astype(
        np.float32
    )
    f = np.outer(np.arange(T, dtype=np.float32), inv)
    cos32 = np.cos(f).T.astype(np.float32)
    sin32 = np.sin(f).T.astype(np.float32)
    cos128 = np.ascontiguousarray(np.concatenate([cos32] * 4, 0)).astype(BF16)
    sin128 = np.ascontiguousarray(np.concatenate([sin32] * 4, 0)).astype(BF16)
    wkv_bw = (wkv_b * kv_norm_w[:, None]).astype(np.float32)
    xt = np.ascontiguousarray(x.reshape(B * T, D).astype(BF16).T)
    wo_c = np.ascontiguousarray(wo).astype(BF16)
    wq_r = wq.reshape(D, H, QKD)

    in_maps = []
    for c in range(NCORES):
        h0 = HPC * c
        w1 = np.concatenate(
            [
                wq_r[:, h0, :NOPE] * scale,
                wq_r[:, h0 + 1, :NOPE] * scale,
                wq_r[:, h0, NOPE:] * scale,
                wq_r[:, h0 + 1, NOPE:] * scale,
                wkv_a[:, c * KVS : (c + 1) * KVS],
            ],
            axis=1,
        ).astype(BF16)
        wb = np.ascontiguousarray(
            wkv_bw[:, h0 * (NOPE + VD) : (h0 + 2) * (NOPE + VD)]
        ).astype(BF16)
        in_maps.append(
            {
                "xt": xt,
                "w1": np.ascontiguousarray(w1),
                "wb": wb,
                "wo": wo_c,
                "cos": cos128,
                "sin": sin128,
            }
        )
    return in_maps


_NC = None


def kernel(x, wq, wkv_a, wkv_b, wo, kv_norm_w, _trace=False):
    global _NC
    if _NC is None:
        _NC = build_program()
    in_maps = host_prep(
        np.asarray(x, np.float32),
        np.asarray(wq, np.float32),
        np.asarray(wkv_a, np.float32),
        np.asarray(wkv_b, np.float32),
        np.asarray(wo, np.float32),
        np.asarray(kv_norm_w, np.float32),
    )
    res = run_bass_kernel_spmd(_NC, in_maps, list(range(NCORES)), trace=_trace)
    out = np.empty((B, T, D), np.float32)
    cw = T // NCORES
    for c in range(NCORES):
        oc = res.results[c]["out"]  # (B, 256, D)
        for b in range(B):
            out[b, c * cw : (c + 1) * cw, :] = oc[b]
    kernel.last_results = res
    return out
